# revision 63
# baseline (speedup 1.0000x reference)
"""Beltrami transformer block on 8 Trainium2 NeuronCores (Bass/Tile).

Shapes (hardcoded per spec): x (4,192,256,256) f32, bpe (4,32,256,256) f32.
B=4, C=192, H=W=256, HEADS=6, d=32, WS=8, K=32.

Sharding: data-parallel over H rows -- core s owns rows [32s, 32s+32).
Windows are 8x8 so shards are fully independent.

Per-core layout: feature-major activations ([channels, tokens] in SBUF).
A "strip" is 8 h-rows x 256 w = 2048 tokens; 16 strips per core; each strip
is processed in four 512-token subtiles.

v2 design notes (vs the DMA-transpose baseline):
- All PSUM tiles are single-bank [128,512] chunks drawn from rotating pools
  (mm x4, avA/avB x2 each) so matmul stages pipeline instead of serializing
  on one 6-bank monolith.
- Attention values are computed FEATURE-major: av = v_sb^T @ e per
  (head, 2-win block) with lhsT = v (token-major [128,33]), so the output
  [33, n] lands channel-on-partition and NO transpose back is needed.
  Requires exp(scores) to be zero on cross-window quadrants: exp is emitted
  as 2 masked ACT ops per head writing only the valid quadrants; the e
  tiles' cross quadrants are zeroed once at startup (persistent e0/e1
  double buffer).
- Softmax denominator rides as an all-ones 33rd v column -> row 32 of each
  head's av block; gpsimd partition_broadcast replicates it over the head's
  33 rows, reciprocal_approx_fast + multiply normalize in feature-major.
- proj weights are host-side permuted to the (head,33)-row layout with zero
  rows at the denominator positions.
- LayerNorm affine + mean-centering folded into the qkv / fc1 weights
  host-side; device computes r = rsqrt(var) via Ln/Exp (natural_log_exp
  table) and multiplies.
- ACT table discipline: everything except gelu lives in natural_log_exp;
  gelu runs are chained per strip and anchor the next strip's first ACT.

This toolchain's walrus rejects >1 sync-wait per instruction; waits are
collapsed by pinning all SW/HW DMA accounting to one FIFO lane each and by a
post-pass that hoists excess waits onto inserted NoOps (see fix_waits).
"""

import numpy as np
import ml_dtypes

B, C, H, W = 4, 192, 256, 256
HEADS, D, WS, KBPE = 6, 32, 8, 32
NCORES = 8
HSH = H // NCORES          # 32 h-rows per core
NSTRIP = B * (HSH // WS)   # 16 strips of 8 rows x 256 w
TOK = WS * W               # 2048 tokens per strip
NSUB = 4                   # 512-token subtiles per strip
SUB = TOK // NSUB          # 512
NBLK = 4                   # 2-window blocks per subtile
EPS = 1e-5

_F32 = np.float32
_BF16 = ml_dtypes.bfloat16
_F8 = ml_dtypes.float8_e4m3

# power-of-2 weight scales so fp8e4m3 sees well-ranged values; matching
# 1/scale is applied at each psum evacuation
WQK_SC = 128.0
WV_SC = 64.0
WFC1_SC = 64.0
WFC2_SC = 64.0
WPROJ_SC = 64.0


def _pack_dr(w, kp):
    """[K, N...] -> DoubleRow [kp, 2, N...]: slot j holds rows kp*j + p."""
    K = w.shape[0]
    assert K == 2 * kp
    return np.ascontiguousarray(
        w.reshape(2, kp, *w.shape[1:]).transpose(1, 0, *range(2, w.ndim + 1)))




# ---- inlined wait-fix post-pass (walrus here caps sync-waits per instruction) ----

from concourse import mybir

MAX_WAITS = 1
_ctr = [0]


def _limit(inst):
    from concourse import mybir as _mb
    n = type(inst).__name__
    if (inst.engine == _mb.EngineType.Pool or "DMA" in n or "Dma" in n
            or "NoOp" in n):
        return 1
    return MAX_WAITS


def fix_waits(nc, verbose=False):
    total_nops = 0
    for fn in nc.m.functions:
        new_blocks = []
        changed = False
        for bb in fn.blocks:
            insts = bb.instructions
            out = []
            for inst in insts:
                si = inst.sync_info
                ws = list(si.on_wait) if si is not None else []
                lim = _limit(inst)
                if len(ws) > lim:
                    movable = [w for w in ws if w.wait_mode == "sem-ge-imm"]
                    fixed = [w for w in ws if w.wait_mode != "sem-ge-imm"]
                    assert len(fixed) <= lim, f"{inst.name}: non-imm waits"
                    keep_n = lim - len(fixed)
                    keep = movable[len(movable) - keep_n:] if keep_n > 0 else []
                    hoist = movable[: len(movable) - keep_n]
                    chunk = 1
                    for i in range(0, len(hoist), chunk):
                        _ctr[0] += 1
                        nop = mybir.InstNoOp(name=f"I-WFIX-{_ctr[0]}")
                        nop.engine = inst.engine
                        nop.bass_nofuse = True
                        nsi = mybir.SyncInfo(on_wait=hoist[i:i + chunk],
                                             on_update=[])
                        nop.sync_info = nsi
                        out.append(nop)
                        total_nops += 1
                    si.on_wait = fixed + keep
                    inst.sync_info = si
                    changed = True
                out.append(inst)
            nb = mybir.BasicBlock(name=bb.name, instructions=out)
            for attr in ("IsExit", "IsLoopEntry", "IsPredicated",
                         "allow_debug_callback_branching",
                         "enter_debug_callback"):
                try:
                    setattr(nb, attr, getattr(bb, attr))
                except Exception:
                    pass
            new_blocks.append(nb)
        if changed:
            fn.blocks = new_blocks
    if verbose:
        print(f"fix_waits: inserted {total_nops} wait-nops")
    return total_nops


def fix_sem_range_clear(nc):
    """This walrus can't codegen EVENT_SEMAPHORE_RANGE_CLEAR (InstISA);
    replace with per-sem EventSemaphore sem-wr-imm 0 writes (same engine,
    same queue position, identical semantics for mode=SEMAPHORE_ZERO)."""
    n = 0
    for fn in nc.m.functions:
        new_blocks = []
        any_changed = False
        for bb in fn.blocks:
            out = []
            changed = False
            for inst in bb.instructions:
                if (type(inst).__name__ == "InstISA"
                        and inst.ant_dict is not None
                        and "range_first" in inst.ant_dict):
                    d = inst.ant_dict
                    assert d.get("mode") == 1, d
                    waits = list(inst.sync_info.on_wait) if inst.sync_info else []
                    upds = list(inst.sync_info.on_update) if inst.sync_info else []
                    ids = range(d["range_first"], d["range_last"] + 1)
                    for k, sem in enumerate(ids):
                        _ctr[0] += 1
                        ev = mybir.InstEventSemaphore(name=f"I-SRC-{_ctr[0]}")
                        ev.engine = inst.engine
                        u = mybir.SyncUpdate(
                            id=sem, update_mode="sem-wr-imm", update_value=0,
                            sync_type="semaphore")
                        ev.sync_info = mybir.SyncInfo(
                            on_wait=waits if k == 0 else [],
                            on_update=[u] + (upds if k == len(ids) - 1 else []))
                        out.append(ev)
                        n += 1
                    changed = True
                else:
                    out.append(inst)
            if changed:
                any_changed = True
                nb = mybir.BasicBlock(name=bb.name, instructions=out)
                for attr in ("IsExit", "IsLoopEntry", "IsPredicated",
                             "allow_debug_callback_branching",
                             "enter_debug_callback"):
                    try:
                        setattr(nb, attr, getattr(bb, attr))
                    except Exception:
                        pass
                new_blocks.append(nb)
            else:
                new_blocks.append(bb)
        if any_changed:
            fn.blocks = new_blocks
    return n


def audit_waits(nc):
    bad = []
    for fn in nc.m.functions:
        for bb in fn.blocks:
            for inst in bb.instructions:
                si = inst.sync_info
                if si is not None and len(si.on_wait) > _limit(inst):
                    bad.append(str(inst)[:150])
    return bad


def _prep_weights(ln1_w, ln1_b, qkv_w, qkv_b, bpe_w, bpe_b, proj_w, proj_b,
                  ln2_w, ln2_b, fc1_w, fc1_b, fc2_w, fc2_b):
    for b_, nm in ((qkv_b, "qkv_b"), (bpe_b, "bpe_b"), (proj_b, "proj_b"),
                   (fc1_b, "fc1_b"), (fc2_b, "fc2_b"), (ln1_b, "ln1_b"),
                   (ln2_b, "ln2_b")):
        assert not np.any(b_), f"{nm} nonzero; bias path not implemented"
    cen = np.eye(C, dtype=np.float64) - 1.0 / C
    w1 = cen @ (np.diag(ln1_w.astype(np.float64)) @ qkv_w.astype(np.float64))
    scale = D ** -0.5
    # wqk: [224, 768] cols = [q'_h0(64) .. q'_h5 | k'_h0 .. k'_h5]
    wqk = np.zeros((C + KBPE, 2 * HEADS * 2 * D), dtype=np.float64)
    bw = bpe_w.astype(np.float64)
    for h in range(HEADS):
        qc, kc = h * D, C + h * D
        bqc, bkc = h * D, HEADS * D + h * D
        base = h * 2 * D
        wqk[:C, base:base + D] = w1[:, qc:qc + D] * scale
        wqk[C:, base + D:base + 2 * D] = bw[:, bqc:bqc + D] * scale
        kb = HEADS * 2 * D + h * 2 * D
        wqk[:C, kb:kb + D] = w1[:, kc:kc + D]
        wqk[C:, kb + D:kb + 2 * D] = bw[:, bkc:bkc + D]
    # wv: [192, 198] cols h*33..h*33+31 = v-cols of head h; col h*33+32 = 0
    # (the 33rd column becomes the softmax denominator via an on-device
    # ones-memset)
    wv = np.zeros((C, HEADS * (D + 1)), dtype=np.float64)
    for h in range(HEADS):
        wv[:, h * (D + 1):h * (D + 1) + D] = w1[:, 2 * C + h * D:2 * C + (h + 1) * D]
    w2 = cen @ (np.diag(ln2_w.astype(np.float64)) @ fc1_w.astype(np.float64))

    # DoubleRow packing with contraction padded to 256 (kp=128) so every
    # on-device producer op is partition-aligned; the zero-padded weight rows
    # kill the dummy lanes.
    def pad256(w):
        out = np.zeros((256, w.shape[1]), dtype=np.float64)
        out[:w.shape[0]] = w
        return out

    wqk_dr = _pack_dr(pad256(wqk * WQK_SC).astype(_F8), 128)   # [128,2,768]
    wv_dr = _pack_dr(pad256(wv * WV_SC).astype(_F8), 128)      # [128,2,198]
    wfc1_dr = _pack_dr(pad256(w2 * WFC1_SC).astype(_F8), 128)  # [128,2,768]
    wfc2_f8 = (fc2_w.astype(np.float64) * WFC2_SC).astype(_F8)
    return (wqk_dr, wv_dr, proj_w.astype(_BF16), wfc1_dr, wfc2_f8)


def _build_nc(reps=1):
    import concourse.bass as bass
    import concourse.tile as tile
    from concourse import mybir
    from concourse.tile_rust import add_dep_helper
    import concourse.tile_sem_assignment as tsa

    # collapse DMA sem accounting to single FIFO lanes (walrus 1-wait limit)
    tsa.NUM_HWDGE_SEMS = 1
    if not getattr(tsa.TileClockTick, "_ant_patched", False):
        _orig = tsa.TileClockTick.__init__

        def _patched(self, *a, **k):
            _orig(self, *a, **k)
            self.swdge_sem_count = 1
        tsa.TileClockTick.__init__ = _patched
        tsa.TileClockTick._ant_patched = True

    dt = mybir.dt
    BF, F32, F8 = dt.bfloat16, dt.float32, dt.float8e4
    AF = mybir.ActivationFunctionType
    ALU = mybir.AluOpType
    DR = mybir.MatmulPerfMode.DoubleRow

    nc = bass.Bass("TRN2", target_bir_lowering=False, debug=False)
    x_d = nc.dram_tensor("x", [B, C, HSH, W], BF, kind="ExternalInput").ap()
    bpe_d = nc.dram_tensor("bpe", [B, KBPE, HSH, W], BF, kind="ExternalInput").ap()
    wqk_d = nc.dram_tensor("wqk", [128, 2, 768], F8, kind="ExternalInput").ap()
    wv_d = nc.dram_tensor("wv", [128, 2, 198], F8, kind="ExternalInput").ap()
    wproj_d = nc.dram_tensor("wproj", [C, C], BF, kind="ExternalInput").ap()
    ident_d = nc.dram_tensor("ident", [128, 128], BF, kind="ExternalInput").ap()
    wfc1_d = nc.dram_tensor("wfc1", [128, 2, 768], F8, kind="ExternalInput").ap()
    wfc2_d = nc.dram_tensor("wfc2", [4 * C, C], F8, kind="ExternalInput").ap()
    wones_d = nc.dram_tensor("wones", [C, 128], BF, kind="ExternalInput").ap()
    out_d = nc.dram_tensor("out", [B, C, HSH, W], F32, kind="ExternalOutput").ap()

    with tile.TileContext(nc) as tc:
        import contextlib
        with contextlib.ExitStack() as ctx:
            singles = ctx.enter_context(tc.tile_pool(name="singles", bufs=1))
            strip_p = ctx.enter_context(tc.tile_pool(name="strip", bufs=2))
            smid_p = ctx.enter_context(tc.tile_pool(name="smid", bufs=1))
            sub_p = ctx.enter_context(tc.tile_pool(name="sub", bufs=3))
            mm_p = ctx.enter_context(
                tc.tile_pool(name="mmp", bufs=4, space="PSUM"))
            av_p = ctx.enter_context(
                tc.tile_pool(name="avp", bufs=2, space="PSUM"))

            # ---- weights to SBUF (once) ----
            wqk_s = singles.tile([128, 2, 768], F8)
            nc.sync.dma_start(out=wqk_s[:], in_=wqk_d[:, :, :])
            wv_s = singles.tile([128, 2, 198], F8)
            nc.sync.dma_start(out=wv_s[:], in_=wv_d[:, :, :])
            ident = singles.tile([128, 128], BF)
            nc.sync.dma_start(out=ident[:], in_=ident_d[:, :])
            wproj1 = singles.tile([128, C], BF)
            nc.sync.dma_start(out=wproj1[:], in_=wproj_d[0:128, :])
            wproj2 = singles.tile([64, C], BF)
            nc.sync.dma_start(out=wproj2[:], in_=wproj_d[128:192, :])
            wfc1_s = singles.tile([128, 2, 768], F8)
            nc.sync.dma_start(out=wfc1_s[:], in_=wfc1_d[:, :, :])
            wfc2s = singles.tile([128, 6, C], F8)
            nc.sync.dma_start(
                out=wfc2s[:], in_=wfc2_d.rearrange("(a p) c -> p a c", p=128))
            wones1 = singles.tile([128, 128], BF)
            nc.sync.dma_start(out=wones1[:], in_=wones_d[0:128, :])
            wones2 = singles.tile([64, 128], BF)
            nc.sync.dma_start(out=wones2[:], in_=wones_d[128:192, :])
            eps_t = singles.tile([128, 1], F32)
            nc.vector.memset(eps_t[:], EPS)

            prev_act = None  # ACT-stream ordering anchor across strips

            def ln_r(xa, xb, tag):
                """x chunks [128,8,256]+[64,8,256] bf16 -> r [128,2048] bf16.
                Returns (r_tile, first_act_inst, last_act_inst)."""
                var_s = smid_p.tile([128, TOK], F32, tag=f"var{tag}")
                first_act = None
                for s in range(NSUB):
                    sl = (slice(None), slice(None), slice(64 * s, 64 * s + 64))
                    sq1 = sub_p.tile([128, 8, 64], BF, tag="sq1")
                    nc.vector.tensor_tensor(sq1[:], xa[sl], xa[sl], ALU.mult)
                    sq2 = sub_p.tile([64, 8, 64], BF, tag="sq2")
                    nc.vector.tensor_tensor(sq2[:], xb[sl], xb[sl], ALU.mult)
                    st0 = mm_p.tile([128, 512], F32, tag="mm")
                    st0v = st0[:].rearrange("p (r w) -> p r w", r=8)
                    nc.tensor.matmul(st0v, wones1[:], xa[sl], start=True, stop=False)
                    nc.tensor.matmul(st0v, wones2[:], xb[sl], start=False, stop=True)
                    st1 = mm_p.tile([128, 512], F32, tag="mm")
                    st1v = st1[:].rearrange("p (r w) -> p r w", r=8)
                    nc.tensor.matmul(st1v, wones1[:], sq1[:],
                                     start=True, stop=False)
                    nc.tensor.matmul(st1v, wones2[:], sq2[:],
                                     start=False, stop=True)
                    m2 = sub_p.tile([128, 512], F32, tag="m2")
                    a = nc.scalar.activation(m2[:], st0[:], AF.Square)
                    if first_act is None:
                        first_act = a
                    nc.vector.tensor_tensor(
                        var_s[:, 512 * s:512 * (s + 1)], st1[:], m2[:],
                        ALU.subtract)
                # per-subtile Ln/Exp chunks so downstream consumers of r
                # chunk s don't wait on the whole strip's variance
                r_t = smid_p.tile([128, TOK], BF, tag=f"r{tag}")
                last = None
                for s in range(NSUB):
                    cs = slice(512 * s, 512 * (s + 1))
                    nc.scalar.activation(var_s[:, cs], var_s[:, cs], AF.Ln,
                                         bias=eps_t[:])
                    last = nc.scalar.activation(r_t[:, cs], var_s[:, cs],
                                                AF.Exp, scale=-0.5)
                return r_t, first_act, last

            import contextlib as _ctl
            rep_cm = (tc.For_i(0, reps, name="reps") if reps > 1
                      else _ctl.nullcontext())
            def load_strip(istrip):
                b, hb = istrip // (HSH // WS), istrip % (HSH // WS)
                rows = slice(hb * WS, hb * WS + WS)
                xa = strip_p.tile([128, 8, 256], BF, tag="xa", name="xa")
                nc.sync.dma_start(out=xa[:], in_=x_d[b, 0:128, rows, :])
                xb = strip_p.tile([64, 8, 256], BF, tag="xb", name="xb")
                nc.sync.dma_start(out=xb[:], in_=x_d[b, 128:192, rows, :])
                # bpe parked at partitions 64:96 so the fp8 DoubleRow slot-1
                # copy is partition-aligned
                bpe_s = strip_p.tile([96, 8, 256], BF, tag="bpe", name="bpe_s")
                nc.sync.dma_start(out=bpe_s[64:96], in_=bpe_d[b, :, rows, :])
                return xa, xb, bpe_s

            with rep_cm:
              pend = load_strip(0)
              for istrip in range(NSTRIP):
                  b, hb = istrip // (HSH // WS), istrip % (HSH // WS)
                  rows = slice(hb * WS, hb * WS + WS)

                  # current strip's inputs were issued BEFORE the previous
                  # strip's out-DMAs (all on the SP queue) so prefetch isn't
                  # blocked behind compute
                  xa, xb, bpe_s = pend
                  if istrip + 1 < NSTRIP:
                      pend = load_strip(istrip + 1)

                  out1 = strip_p.tile([128, 8, 256], F32, tag="out1")
                  out2 = strip_p.tile([64, 8, 256], F32, tag="out2")
                  xh1 = smid_p.tile([128, 8, 256], BF, tag="xh1")
                  xh2_ = smid_p.tile([64, 8, 256], BF, tag="xh2")

                  # ---------- LN1 r ----------
                  r1, fa, la = ln_r(xa, xb, "1")
                  if prev_act is not None:
                      add_dep_helper(fa.ins, prev_act.ins, sync=False,
                                     reason="act-set order")

                  # ---------- attention + proj + resid, per subtile ----------
                  for s in range(NSUB):
                      sl = (slice(None), slice(None), slice(64 * s, 64 * s + 64))
                      WM_R = "p (r g v w) -> p g v r w"   # row-major -> win-major
                      WM_X = "p r (g v w) -> p g v r w"
                      WM_O = "p (g v r w) -> p g v r w"
                      rwm = r1[:, 512 * s:512 * (s + 1)].rearrange(
                          WM_R, g=4, v=2, r=8)
                      # xr in fp8 DoubleRow layout [128, 2, 512], contraction
                      # padded to 256: slot0 = x[0:128]; slot1 = [x 128:192,
                      # bpe at 64:96, dummy 96:128 (zero weight rows)];
                      # win-major token order: col = 128*blk + 64*win + 8*hr + w
                      WM_O2 = "p (g v r w) -> p g v r w"
                      xr = sub_p.tile([128, 2, 512], F8, tag="xr")
                      nc.vector.tensor_tensor(
                          xr[:, 0].rearrange(WM_O2, g=4, v=2, r=8),
                          xa[sl].rearrange(WM_X, g=4, v=2),
                          rwm, ALU.mult)
                      nc.vector.tensor_tensor(
                          xr[0:64, 1].rearrange(WM_O2, g=4, v=2, r=8),
                          xb[sl].rearrange(WM_X, g=4, v=2),
                          rwm[0:64], ALU.mult)
                      nc.gpsimd.tensor_copy(
                          out=xr[64:96, 1].rearrange(WM_O2, g=4, v=2, r=8),
                          in_=bpe_s[64:96, :, 64 * s:64 * s + 64].rearrange(
                              WM_X, g=4, v=2))
                      nc.gpsimd.memset(xr[96:128, 1], 0.0)

                      # q'k' matmuls: 6 fp8 DoubleRow chunks, 1-bank psum each.
                      # Order 0,3,1,4,2,5 so head 0/1 scores (chunks 0+3) can
                      # start after only two copies land.
                      qk_sb = sub_p.tile([128, 6, 512], BF, tag="qksb")
                      for i, cc in enumerate((0, 3, 1, 4, 2, 5)):
                          qp = mm_p.tile([128, 512], F32, tag="mm")
                          nc.tensor.matmul(qp[:],
                                           wqk_s[:, :, 128 * cc:128 * (cc + 1)],
                                           xr[:], start=True, stop=True,
                                           perf_mode=DR)
                          if i % 2 == 0:
                              nc.vector.tensor_scalar_mul(
                                  out=qk_sb[:, cc], in0=qp[:],
                                  scalar1=1.0 / WQK_SC)
                          else:
                              nc.scalar.activation(qk_sb[:, cc], qp[:],
                                                   AF.Copy, scale=1.0 / WQK_SC)

                      # v (token-major, win-major order); 33rd col per head is
                      # the ones column -> softmax denominator rides the av
                      v_sb = sub_p.tile([128, 4, 6, 33], BF, tag="vsb")
                      for vh in range(2):
                          vp = mm_p.tile([128, 512], F32, tag="mm")
                          vpv = vp[:].rearrange("p (a n) -> p a n", a=2)
                          for j in range(2):
                              blk = 2 * vh + j
                              bsl = slice(128 * blk, 128 * (blk + 1))
                              nc.tensor.matmul(vpv[:, j, 0:198],
                                               xr[:, :, bsl], wv_s[:],
                                               start=True, stop=True,
                                               perf_mode=DR)
                          nc.vector.tensor_scalar_mul(
                              out=v_sb[:, 2 * vh:2 * vh + 2],
                              in0=vpv[:, :, 0:198].rearrange(
                                  "p a (h e) -> p a h e", h=6),
                              scalar1=1.0 / WV_SC)
                      nc.gpsimd.memset(v_sb[:, :, :, 32:33], 1.0)

                      # scores per head -> exp (full tile; cross-window
                      # quadrants hold garbage that av never reads)
                      e_t = sub_p.tile([128, 6, 512], BF, tag="et")
                      for h in range(HEADS):
                          ro = 64 * (h % 2)
                          qch, kch = h // 2, 3 + h // 2
                          sc = mm_p.tile([128, 512], F32, tag="mm")
                          for blk in range(NBLK):
                              bsl = slice(128 * blk, 128 * (blk + 1))
                              nc.tensor.matmul(
                                  sc[:, bsl],
                                  qk_sb[ro:ro + 64, kch, bsl],
                                  qk_sb[ro:ro + 64, qch, bsl],
                                  start=True, stop=True)
                          nc.scalar.activation(e_t[:, h], sc[:], AF.Exp)

                      # av token-major: out[n, 33] per (head, blk, win) from
                      # the window-diagonal e slices
                      avt = []
                      for i in range(2):
                          av_t = mm_p.tile([128, 512], F32, tag="mm",
                                           name=f"avt{i}")
                          avt.append(av_t[:].rearrange("p (a n) -> p a n", a=2))
                      for h in range(HEADS):
                          for blk in range(NBLK):
                              for wn in range(2):
                                  po = 64 * wn
                                  nc.tensor.matmul(
                                      avt[blk // 2][po:po + 64, blk % 2,
                                                    33 * h:33 * h + 33],
                                      e_t[po:po + 64, h,
                                          128 * blk + po:128 * blk + po + 64],
                                      v_sb[po:po + 64, blk, h],
                                      start=True, stop=True)

                      # normalize in token-major (reciprocal over 12 denoms
                      # per partition), then PE-transpose back to feature-major
                      att = sub_p.tile([128, 4, 256], BF, tag="att")
                      for i in range(2):
                          avv = avt[i][:, :, 0:198].rearrange(
                              "p a (h e) -> p a h e", h=6)
                          rec = sub_p.tile([128, 2, 6, 1], F32, tag=f"rec{i}",
                                           name=f"rec{i}")
                          nc.vector.reciprocal(rec[:], avv[:, :, :, 32:33])
                          nc.vector.tensor_tensor(
                              att[:, 2 * i:2 * i + 2, 0:192].rearrange(
                                  "p a (h e) -> p a h e", h=6),
                              avv[:, :, :, 0:32],
                              rec[:].to_broadcast([128, 2, 6, 32]), ALU.mult)

                      at1_ps = av_p.tile([128, 512], BF, tag="atp1")
                      at2_pt = av_p.tile([64, 512], BF, tag="atp2")
                      at2_ps = at2_pt[0:64, :]
                      for blk in range(NBLK):
                          bsl = slice(128 * blk, 128 * (blk + 1))
                          nc.tensor.transpose(at1_ps[:, bsl],
                                              att[:, blk, 0:128], ident[:])
                          nc.tensor.transpose(at2_ps[:, bsl],
                                              att[:, blk, 128:192],
                                              ident[:])
                      at1 = sub_p.tile([128, 512], BF, tag="at1")
                      nc.vector.tensor_copy(out=at1[:], in_=at1_ps[:])
                      at2 = sub_p.tile([64, 512], BF, tag="at2")
                      nc.scalar.activation(at2[:], at2_ps, AF.Copy)

                      # proj
                      pr = mm_p.tile([128, 512], F32, tag="mm")
                      pr2t = mm_p.tile([128, 512], F32, tag="mm")
                      pr2 = pr2t[0:64, :]
                      nc.tensor.matmul(pr[:], wproj1[:, 0:128], at1[:],
                                       start=True, stop=False)
                      nc.tensor.matmul(pr[:], wproj2[:, 0:128], at2[:],
                                       start=False, stop=True)
                      nc.tensor.matmul(pr2, wproj1[:, 128:192], at1[:],
                                       start=True, stop=False)
                      nc.tensor.matmul(pr2, wproj2[:, 128:192], at2[:],
                                       start=False, stop=True)

                      # residual 1 (block-token order -> row-major reorder)
                      ord1 = pr[:].rearrange("p (g v r w) -> p r g v w",
                                             g=4, v=2, r=8)
                      ord2 = pr2.rearrange("p (g v r w) -> p r g v w",
                                           g=4, v=2, r=8)
                      x5 = "p r (g v w) -> p r g v w"
                      nc.vector.tensor_tensor(
                          xh1[sl].rearrange(x5, g=4, v=2),
                          xa[sl].rearrange(x5, g=4, v=2), ord1, ALU.add)
                      nc.vector.tensor_tensor(
                          xh2_[sl].rearrange(x5, g=4, v=2),
                          xb[sl].rearrange(x5, g=4, v=2), ord2, ALU.add)

                  # ---------- LN2 r ----------
                  r2, fa2, la2 = ln_r(xh1, xh2_, "2")
                  add_dep_helper(fa2.ins, la.ins, sync=False,
                                 reason="act-set order")

                  # ---------- MLP per subtile ----------
                  prev_gelu = la2
                  for s in range(NSUB):
                      sl = (slice(None), slice(None), slice(64 * s, 64 * s + 64))
                      rsl = r2[:, 512 * s:512 * (s + 1)].rearrange(
                          "p (r w) -> p r w", r=8)
                      # y in fp8 DoubleRow [128, 2, 512], contraction padded
                      # to 256: slot0 = ch 0:128; slot1 = [ch 128:192, dummy]
                      y = sub_p.tile([128, 2, 512], F8, tag="y")
                      RW = "p (r w) -> p r w"
                      nc.vector.tensor_tensor(
                          y[:, 0].rearrange(RW, r=8), xh1[sl], rsl, ALU.mult)
                      nc.vector.tensor_tensor(
                          y[0:64, 1].rearrange(RW, r=8),
                          xh2_[sl], rsl[0:64], ALU.mult)
                      nc.gpsimd.memset(y[64:128, 1], 0.0)

                      h_sb = sub_p.tile([128, 6, 512], F8, tag="hsb")
                      for cc in range(6):
                          fp = mm_p.tile([128, 512], F32, tag="mm")
                          nc.tensor.matmul(fp[:],
                                           wfc1_s[:, :, 128 * cc:128 * (cc + 1)],
                                           y[:], start=True, stop=True,
                                           perf_mode=DR)
                          g = nc.scalar.activation(h_sb[:, cc], fp[:], AF.Gelu,
                                                   scale=1.0 / WFC1_SC)
                          add_dep_helper(g.ins, prev_gelu.ins, sync=False,
                                         reason="act-set order")
                          prev_gelu = g

                      f2 = mm_p.tile([128, 512], F32, tag="mm")
                      f22t = mm_p.tile([128, 512], F32, tag="mm")
                      f22 = f22t[0:64, :]
                      for i in range(3):
                          nc.tensor.matmul(f2[:], wfc2s[:, 2 * i:2 * i + 2, 0:128],
                                           h_sb[:, 2 * i:2 * i + 2],
                                           start=(i == 0), stop=(i == 2),
                                           perf_mode=DR)
                      for i in range(3):
                          nc.tensor.matmul(f22, wfc2s[:, 2 * i:2 * i + 2, 128:192],
                                           h_sb[:, 2 * i:2 * i + 2],
                                           start=(i == 0), stop=(i == 2),
                                           perf_mode=DR)

                      nc.vector.scalar_tensor_tensor(
                          out1[sl], f2[:].rearrange("p (r w) -> p r w", r=8),
                          1.0 / WFC2_SC, xh1[sl], ALU.mult, ALU.add)
                      nc.vector.scalar_tensor_tensor(
                          out2[sl], f22.rearrange("p (r w) -> p r w", r=8),
                          1.0 / WFC2_SC, xh2_[sl], ALU.mult, ALU.add)
                  prev_act = prev_gelu

                  nc.sync.dma_start(out=out_d[b, 0:128, rows, :], in_=out1[:])
                  nc.sync.dma_start(out=out_d[b, 128:192, rows, :], in_=out2[:])

    fix_sem_range_clear(nc)
    fix_waits(nc)
    bad = audit_waits(nc)
    assert not bad, f"wait audit: {len(bad)} violations: {bad[:3]}"
    return nc


_CACHED = None


def _get_nc():
    global _CACHED
    if _CACHED is None:
        _CACHED = _build_nc()
    return _CACHED


def _make_in_maps(x, bpe_encodings, weights):
    wqk, wv, wproj, wfc1, wfc2 = weights
    wones = np.full((C, 128), 1.0 / C, dtype=_BF16)
    ident = np.eye(128, dtype=_BF16)
    xb = np.asarray(x, _F32).astype(_BF16)
    bb = np.asarray(bpe_encodings, _F32).astype(_BF16)
    in_maps = []
    for s in range(NCORES):
        rows = slice(s * HSH, (s + 1) * HSH)
        in_maps.append({
            "x": np.ascontiguousarray(xb[:, :, rows, :]),
            "bpe": np.ascontiguousarray(bb[:, :, rows, :]),
            "wqk": wqk, "wv": wv, "wproj": wproj,
            "wfc1": wfc1, "wfc2": wfc2, "wones": wones, "ident": ident,
        })
    return in_maps


def kernel(x, bpe_encodings, ln1_w, ln1_b, qkv_w, qkv_b, bpe_w, bpe_b,
           proj_w, proj_b, ln2_w, ln2_b, fc1_w, fc1_b, fc2_w, fc2_b,
           **_kw):
    from concourse.bass_utils import run_bass_kernel_spmd

    weights = _prep_weights(
        np.asarray(ln1_w, _F32), np.asarray(ln1_b, _F32),
        np.asarray(qkv_w, _F32), np.asarray(qkv_b, _F32),
        np.asarray(bpe_w, _F32), np.asarray(bpe_b, _F32),
        np.asarray(proj_w, _F32), np.asarray(proj_b, _F32),
        np.asarray(ln2_w, _F32), np.asarray(ln2_b, _F32),
        np.asarray(fc1_w, _F32), np.asarray(fc1_b, _F32),
        np.asarray(fc2_w, _F32), np.asarray(fc2_b, _F32))
    in_maps = _make_in_maps(x, bpe_encodings, weights)

    nc = _get_nc()
    res = run_bass_kernel_spmd(nc, in_maps, core_ids=list(range(NCORES)))
    out = np.empty((B, C, H, W), dtype=np.float32)
    for s in range(NCORES):
        out[:, :, s * HSH:(s + 1) * HSH, :] = res.results[s]["out"]
    return out


# revision 64
# speedup vs baseline: 1.5525x; 1.5525x over previous
"""Beltrami transformer block on 8 Trainium2 NeuronCores (Bass/Tile).

Shapes (hardcoded per spec): x (4,192,256,256) f32, bpe (4,32,256,256) f32.
B=4, C=192, H=W=256, HEADS=6, d=32, WS=8, K=32.

Sharding: data-parallel over H rows -- core s owns rows [32s, 32s+32).
Windows are 8x8 so shards are fully independent.

Per-core layout: feature-major activations ([channels, tokens] in SBUF).
A "strip" is 8 h-rows x 256 w = 2048 tokens; 16 strips per core; each strip
is processed in four 512-token subtiles.

v2 design notes (vs the DMA-transpose baseline):
- All PSUM tiles are single-bank [128,512] chunks drawn from rotating pools
  (mm x4, avA/avB x2 each) so matmul stages pipeline instead of serializing
  on one 6-bank monolith.
- Attention values are computed FEATURE-major: av = v_sb^T @ e per
  (head, 2-win block) with lhsT = v (token-major [128,33]), so the output
  [33, n] lands channel-on-partition and NO transpose back is needed.
  Requires exp(scores) to be zero on cross-window quadrants: exp is emitted
  as 2 masked ACT ops per head writing only the valid quadrants; the e
  tiles' cross quadrants are zeroed once at startup (persistent e0/e1
  double buffer).
- Softmax denominator rides as an all-ones 33rd v column -> row 32 of each
  head's av block; gpsimd partition_broadcast replicates it over the head's
  33 rows, reciprocal_approx_fast + multiply normalize in feature-major.
- proj weights are host-side permuted to the (head,33)-row layout with zero
  rows at the denominator positions.
- LayerNorm affine + mean-centering folded into the qkv / fc1 weights
  host-side; device computes r = rsqrt(var) via Ln/Exp (natural_log_exp
  table) and multiplies.
- ACT table discipline: everything except gelu lives in natural_log_exp;
  gelu runs are chained per strip and anchor the next strip's first ACT.

This toolchain's walrus rejects >1 sync-wait per instruction; waits are
collapsed by pinning all SW/HW DMA accounting to one FIFO lane each and by a
post-pass that hoists excess waits onto inserted NoOps (see fix_waits).
"""

import numpy as np
import ml_dtypes

B, C, H, W = 4, 192, 256, 256
HEADS, D, WS, KBPE = 6, 32, 8, 32
NCORES = 8
HSH = H // NCORES          # 32 h-rows per core
NSTRIP = B * (HSH // WS)   # 16 strips of 8 rows x 256 w
TOK = WS * W               # 2048 tokens per strip
NSUB = 4                   # 512-token subtiles per strip
SUB = TOK // NSUB          # 512
NBLK = 4                   # 2-window blocks per subtile
EPS = 1e-5

_F32 = np.float32
_BF16 = ml_dtypes.bfloat16
_F8 = ml_dtypes.float8_e4m3

# power-of-2 weight scales so fp8e4m3 sees well-ranged values; matching
# 1/scale is applied at each psum evacuation
WQK_SC = 128.0
WV_SC = 64.0
WFC1_SC = 64.0
WFC2_SC = 64.0
WPROJ_SC = 64.0


def _pack_dr(w, kp):
    """[K, N...] -> DoubleRow [kp, 2, N...]: slot j holds rows kp*j + p."""
    K = w.shape[0]
    assert K == 2 * kp
    return np.ascontiguousarray(
        w.reshape(2, kp, *w.shape[1:]).transpose(1, 0, *range(2, w.ndim + 1)))




# ---- inlined wait-fix post-pass (walrus here caps sync-waits per instruction) ----

from concourse import mybir

MAX_WAITS = 1
_ctr = [0]


def _limit(inst):
    from concourse import mybir as _mb
    n = type(inst).__name__
    if (inst.engine == _mb.EngineType.Pool or "DMA" in n or "Dma" in n
            or "NoOp" in n):
        return 1
    return MAX_WAITS


def fix_waits(nc, verbose=False):
    total_nops = 0
    for fn in nc.m.functions:
        new_blocks = []
        changed = False
        for bb in fn.blocks:
            insts = bb.instructions
            out = []
            for inst in insts:
                si = inst.sync_info
                ws = list(si.on_wait) if si is not None else []
                lim = _limit(inst)
                if len(ws) > lim:
                    movable = [w for w in ws if w.wait_mode == "sem-ge-imm"]
                    fixed = [w for w in ws if w.wait_mode != "sem-ge-imm"]
                    assert len(fixed) <= lim, f"{inst.name}: non-imm waits"
                    keep_n = lim - len(fixed)
                    keep = movable[len(movable) - keep_n:] if keep_n > 0 else []
                    hoist = movable[: len(movable) - keep_n]
                    chunk = 1
                    for i in range(0, len(hoist), chunk):
                        _ctr[0] += 1
                        nop = mybir.InstNoOp(name=f"I-WFIX-{_ctr[0]}")
                        nop.engine = inst.engine
                        nop.bass_nofuse = True
                        nsi = mybir.SyncInfo(on_wait=hoist[i:i + chunk],
                                             on_update=[])
                        nop.sync_info = nsi
                        out.append(nop)
                        total_nops += 1
                    si.on_wait = fixed + keep
                    inst.sync_info = si
                    changed = True
                out.append(inst)
            nb = mybir.BasicBlock(name=bb.name, instructions=out)
            for attr in ("IsExit", "IsLoopEntry", "IsPredicated",
                         "allow_debug_callback_branching",
                         "enter_debug_callback"):
                try:
                    setattr(nb, attr, getattr(bb, attr))
                except Exception:
                    pass
            new_blocks.append(nb)
        if changed:
            fn.blocks = new_blocks
    if verbose:
        print(f"fix_waits: inserted {total_nops} wait-nops")
    return total_nops


def fix_sem_range_clear(nc):
    """This walrus can't codegen EVENT_SEMAPHORE_RANGE_CLEAR (InstISA);
    replace with per-sem EventSemaphore sem-wr-imm 0 writes (same engine,
    same queue position, identical semantics for mode=SEMAPHORE_ZERO)."""
    n = 0
    for fn in nc.m.functions:
        new_blocks = []
        any_changed = False
        for bb in fn.blocks:
            out = []
            changed = False
            for inst in bb.instructions:
                if (type(inst).__name__ == "InstISA"
                        and inst.ant_dict is not None
                        and "range_first" in inst.ant_dict):
                    d = inst.ant_dict
                    assert d.get("mode") == 1, d
                    waits = list(inst.sync_info.on_wait) if inst.sync_info else []
                    upds = list(inst.sync_info.on_update) if inst.sync_info else []
                    ids = range(d["range_first"], d["range_last"] + 1)
                    for k, sem in enumerate(ids):
                        _ctr[0] += 1
                        ev = mybir.InstEventSemaphore(name=f"I-SRC-{_ctr[0]}")
                        ev.engine = inst.engine
                        u = mybir.SyncUpdate(
                            id=sem, update_mode="sem-wr-imm", update_value=0,
                            sync_type="semaphore")
                        ev.sync_info = mybir.SyncInfo(
                            on_wait=waits if k == 0 else [],
                            on_update=[u] + (upds if k == len(ids) - 1 else []))
                        out.append(ev)
                        n += 1
                    changed = True
                else:
                    out.append(inst)
            if changed:
                any_changed = True
                nb = mybir.BasicBlock(name=bb.name, instructions=out)
                for attr in ("IsExit", "IsLoopEntry", "IsPredicated",
                             "allow_debug_callback_branching",
                             "enter_debug_callback"):
                    try:
                        setattr(nb, attr, getattr(bb, attr))
                    except Exception:
                        pass
                new_blocks.append(nb)
            else:
                new_blocks.append(bb)
        if any_changed:
            fn.blocks = new_blocks
    return n


def audit_waits(nc):
    bad = []
    for fn in nc.m.functions:
        for bb in fn.blocks:
            for inst in bb.instructions:
                si = inst.sync_info
                if si is not None and len(si.on_wait) > _limit(inst):
                    bad.append(str(inst)[:150])
    return bad


def _prep_weights(ln1_w, ln1_b, qkv_w, qkv_b, bpe_w, bpe_b, proj_w, proj_b,
                  ln2_w, ln2_b, fc1_w, fc1_b, fc2_w, fc2_b):
    for b_, nm in ((qkv_b, "qkv_b"), (bpe_b, "bpe_b"), (proj_b, "proj_b"),
                   (fc1_b, "fc1_b"), (fc2_b, "fc2_b"), (ln1_b, "ln1_b"),
                   (ln2_b, "ln2_b")):
        assert not np.any(b_), f"{nm} nonzero; bias path not implemented"
    cen = np.eye(C, dtype=np.float64) - 1.0 / C
    w1 = cen @ (np.diag(ln1_w.astype(np.float64)) @ qkv_w.astype(np.float64))
    scale = D ** -0.5
    # wqk: [224, 768] cols = [q'_h0(64) .. q'_h5 | k'_h0 .. k'_h5]
    wqk = np.zeros((C + KBPE, 2 * HEADS * 2 * D), dtype=np.float64)
    bw = bpe_w.astype(np.float64)
    for h in range(HEADS):
        qc, kc = h * D, C + h * D
        bqc, bkc = h * D, HEADS * D + h * D
        base = h * 2 * D
        wqk[:C, base:base + D] = w1[:, qc:qc + D] * scale
        wqk[C:, base + D:base + 2 * D] = bw[:, bqc:bqc + D] * scale
        kb = HEADS * 2 * D + h * 2 * D
        wqk[:C, kb:kb + D] = w1[:, kc:kc + D]
        wqk[C:, kb + D:kb + 2 * D] = bw[:, bkc:bkc + D]
    # wv: [192, 198] cols h*33..h*33+31 = v-cols of head h; col h*33+32 = 0
    # (the 33rd column becomes the softmax denominator via an on-device
    # ones-memset)
    wv = np.zeros((C, HEADS * (D + 1)), dtype=np.float64)
    for h in range(HEADS):
        wv[:, h * (D + 1):h * (D + 1) + D] = w1[:, 2 * C + h * D:2 * C + (h + 1) * D]
    w2 = cen @ (np.diag(ln2_w.astype(np.float64)) @ fc1_w.astype(np.float64))

    # DoubleRow packing with contraction padded to 256 (kp=128) so every
    # on-device producer op is partition-aligned; the zero-padded weight rows
    # kill the dummy lanes.
    def pad256(w):
        out = np.zeros((256, w.shape[1]), dtype=np.float64)
        out[:w.shape[0]] = w
        return out

    wqk_dr = _pack_dr(pad256(wqk * WQK_SC).astype(_F8), 128)   # [128,2,768]
    wv_dr = _pack_dr(pad256(wv * WV_SC).astype(_F8), 128)      # [128,2,198]
    wfc1_dr = _pack_dr(pad256(w2 * WFC1_SC).astype(_F8), 128)  # [128,2,768]
    wfc2_f8 = (fc2_w.astype(np.float64) * WFC2_SC).astype(_F8)
    return (wqk_dr, wv_dr, proj_w.astype(_BF16), wfc1_dr, wfc2_f8)


def _build_nc(reps=1):
    import concourse.bass as bass
    import concourse.tile as tile
    from concourse import mybir
    from concourse.tile_rust import add_dep_helper
    import concourse.tile_sem_assignment as tsa

    # collapse DMA sem accounting to single FIFO lanes (walrus 1-wait limit)
    tsa.NUM_HWDGE_SEMS = 1
    if not getattr(tsa.TileClockTick, "_ant_patched", False):
        _orig = tsa.TileClockTick.__init__

        def _patched(self, *a, **k):
            _orig(self, *a, **k)
            self.swdge_sem_count = 1
        tsa.TileClockTick.__init__ = _patched
        tsa.TileClockTick._ant_patched = True

    dt = mybir.dt
    BF, F32, F8 = dt.bfloat16, dt.float32, dt.float8e4
    AF = mybir.ActivationFunctionType
    ALU = mybir.AluOpType
    DR = mybir.MatmulPerfMode.DoubleRow

    nc = bass.Bass("TRN2", target_bir_lowering=False, debug=False)
    x_d = nc.dram_tensor("x", [B, C, HSH, W], BF, kind="ExternalInput").ap()
    bpe_d = nc.dram_tensor("bpe", [B, KBPE, HSH, W], BF, kind="ExternalInput").ap()
    wqk_d = nc.dram_tensor("wqk", [128, 2, 768], F8, kind="ExternalInput").ap()
    wv_d = nc.dram_tensor("wv", [128, 2, 198], F8, kind="ExternalInput").ap()
    wproj_d = nc.dram_tensor("wproj", [C, C], BF, kind="ExternalInput").ap()
    ident_d = nc.dram_tensor("ident", [128, 128], BF, kind="ExternalInput").ap()
    wfc1_d = nc.dram_tensor("wfc1", [128, 2, 768], F8, kind="ExternalInput").ap()
    wfc2_d = nc.dram_tensor("wfc2", [4 * C, C], F8, kind="ExternalInput").ap()
    wones_d = nc.dram_tensor("wones", [C, 128], BF, kind="ExternalInput").ap()
    out_d = nc.dram_tensor("out", [B, C, HSH, W], F32, kind="ExternalOutput").ap()

    with tile.TileContext(nc) as tc:
        import contextlib
        with contextlib.ExitStack() as ctx:
            singles = ctx.enter_context(tc.tile_pool(name="singles", bufs=1))
            strip_p = ctx.enter_context(tc.tile_pool(name="strip", bufs=2))
            smid_p = ctx.enter_context(tc.tile_pool(name="smid", bufs=1))
            sub_p = ctx.enter_context(tc.tile_pool(name="sub", bufs=3))
            mm_p = ctx.enter_context(
                tc.tile_pool(name="mmp", bufs=4, space="PSUM"))
            av_p = ctx.enter_context(
                tc.tile_pool(name="avp", bufs=2, space="PSUM"))

            # ---- weights to SBUF (once) ----
            wqk_s = singles.tile([128, 2, 768], F8)
            nc.sync.dma_start(out=wqk_s[:], in_=wqk_d[:, :, :])
            wv_s = singles.tile([128, 2, 198], F8)
            nc.sync.dma_start(out=wv_s[:], in_=wv_d[:, :, :])
            ident = singles.tile([128, 128], BF)
            nc.sync.dma_start(out=ident[:], in_=ident_d[:, :])
            wproj1 = singles.tile([128, C], BF)
            nc.sync.dma_start(out=wproj1[:], in_=wproj_d[0:128, :])
            wproj2 = singles.tile([64, C], BF)
            nc.sync.dma_start(out=wproj2[:], in_=wproj_d[128:192, :])
            wfc1_s = singles.tile([128, 2, 768], F8)
            nc.sync.dma_start(out=wfc1_s[:], in_=wfc1_d[:, :, :])
            wfc2s = singles.tile([128, 6, C], F8)
            nc.sync.dma_start(
                out=wfc2s[:], in_=wfc2_d.rearrange("(a p) c -> p a c", p=128))
            wones1 = singles.tile([128, 128], BF)
            nc.sync.dma_start(out=wones1[:], in_=wones_d[0:128, :])
            wones2 = singles.tile([64, 128], BF)
            nc.sync.dma_start(out=wones2[:], in_=wones_d[128:192, :])
            eps_t = singles.tile([128, 1], F32)
            nc.vector.memset(eps_t[:], EPS)

            prev_act = None  # ACT-stream ordering anchor across strips

            def ln_r(xa, xb, tag):
                """x chunks [128,8,256]+[64,8,256] bf16 -> r [128,2048] bf16.
                Returns (r_tile, first_act_inst, last_act_inst)."""
                var_s = smid_p.tile([128, TOK], F32, tag=f"var{tag}")
                first_act = None
                for s in range(NSUB):
                    sl = (slice(None), slice(None), slice(64 * s, 64 * s + 64))
                    sq1 = sub_p.tile([128, 8, 64], BF, tag="sq1")
                    nc.vector.tensor_tensor(sq1[:], xa[sl], xa[sl], ALU.mult)
                    sq2 = sub_p.tile([64, 8, 64], BF, tag="sq2")
                    nc.vector.tensor_tensor(sq2[:], xb[sl], xb[sl], ALU.mult)
                    st0 = mm_p.tile([128, 512], F32, tag="mm")
                    st0v = st0[:].rearrange("p (r w) -> p r w", r=8)
                    nc.tensor.matmul(st0v, wones1[:], xa[sl], start=True, stop=False)
                    nc.tensor.matmul(st0v, wones2[:], xb[sl], start=False, stop=True)
                    st1 = mm_p.tile([128, 512], F32, tag="mm")
                    st1v = st1[:].rearrange("p (r w) -> p r w", r=8)
                    nc.tensor.matmul(st1v, wones1[:], sq1[:],
                                     start=True, stop=False)
                    nc.tensor.matmul(st1v, wones2[:], sq2[:],
                                     start=False, stop=True)
                    m2 = sub_p.tile([128, 512], F32, tag="m2")
                    a = nc.scalar.activation(m2[:], st0[:], AF.Square)
                    if first_act is None:
                        first_act = a
                    nc.vector.tensor_tensor(
                        var_s[:, 512 * s:512 * (s + 1)], st1[:], m2[:],
                        ALU.subtract)
                # per-subtile Ln/Exp chunks so downstream consumers of r
                # chunk s don't wait on the whole strip's variance
                r_t = smid_p.tile([128, TOK], BF, tag=f"r{tag}")
                last = None
                for s in range(NSUB):
                    cs = slice(512 * s, 512 * (s + 1))
                    nc.scalar.activation(var_s[:, cs], var_s[:, cs], AF.Ln,
                                         bias=eps_t[:])
                    last = nc.scalar.activation(r_t[:, cs], var_s[:, cs],
                                                AF.Exp, scale=-0.5)
                return r_t, first_act, last

            import contextlib as _ctl
            rep_cm = (tc.For_i(0, reps, name="reps") if reps > 1
                      else _ctl.nullcontext())
            def load_strip(istrip):
                b, hb = istrip // (HSH // WS), istrip % (HSH // WS)
                rows = slice(hb * WS, hb * WS + WS)
                xa = strip_p.tile([128, 8, 256], BF, tag="xa", name="xa")
                nc.sync.dma_start(out=xa[:], in_=x_d[b, 0:128, rows, :])
                xb = strip_p.tile([64, 8, 256], BF, tag="xb", name="xb")
                nc.sync.dma_start(out=xb[:], in_=x_d[b, 128:192, rows, :])
                # bpe parked at partitions 64:96 so the fp8 DoubleRow slot-1
                # copy is partition-aligned
                bpe_s = strip_p.tile([96, 8, 256], BF, tag="bpe", name="bpe_s")
                nc.sync.dma_start(out=bpe_s[64:96], in_=bpe_d[b, :, rows, :])
                return xa, xb, bpe_s

            with rep_cm:
              pend = load_strip(0)
              for istrip in range(NSTRIP):
                  b, hb = istrip // (HSH // WS), istrip % (HSH // WS)
                  rows = slice(hb * WS, hb * WS + WS)

                  # current strip's inputs were issued BEFORE the previous
                  # strip's out-DMAs (all on the SP queue) so prefetch isn't
                  # blocked behind compute
                  xa, xb, bpe_s = pend
                  if istrip + 1 < NSTRIP:
                      pend = load_strip(istrip + 1)

                  out1 = strip_p.tile([128, 8, 256], F32, tag="out1")
                  out2 = strip_p.tile([64, 8, 256], F32, tag="out2")
                  xh1 = smid_p.tile([128, 8, 256], BF, tag="xh1")
                  xh2_ = smid_p.tile([64, 8, 256], BF, tag="xh2")

                  # ---------- LN1 r ----------
                  r1, fa, la = ln_r(xa, xb, "1")
                  if prev_act is not None:
                      add_dep_helper(fa.ins, prev_act.ins, sync=False,
                                     reason="act-set order")

                  # ---------- attention + proj + resid, per subtile ----------
                  for s in range(NSUB):
                      sl = (slice(None), slice(None), slice(64 * s, 64 * s + 64))
                      WM_R = "p (r g v w) -> p g v r w"   # row-major -> win-major
                      WM_X = "p r (g v w) -> p g v r w"
                      WM_O = "p (g v r w) -> p g v r w"
                      rwm = r1[:, 512 * s:512 * (s + 1)].rearrange(
                          WM_R, g=4, v=2, r=8)
                      # xr in fp8 DoubleRow layout [128, 2, 512], contraction
                      # padded to 256: slot0 = x[0:128]; slot1 = [x 128:192,
                      # bpe at 64:96, dummy 96:128 (zero weight rows)];
                      # win-major token order: col = 128*blk + 64*win + 8*hr + w
                      WM_O2 = "p (g v r w) -> p g v r w"
                      xr = sub_p.tile([128, 2, 512], F8, tag="xr")
                      nc.vector.tensor_tensor(
                          xr[:, 0].rearrange(WM_O2, g=4, v=2, r=8),
                          xa[sl].rearrange(WM_X, g=4, v=2),
                          rwm, ALU.mult)
                      nc.vector.tensor_tensor(
                          xr[0:64, 1].rearrange(WM_O2, g=4, v=2, r=8),
                          xb[sl].rearrange(WM_X, g=4, v=2),
                          rwm[0:64], ALU.mult)
                      nc.vector.tensor_copy(
                          out=xr[64:96, 1].rearrange(WM_O2, g=4, v=2, r=8),
                          in_=bpe_s[64:96, :, 64 * s:64 * s + 64].rearrange(
                              WM_X, g=4, v=2))
                      nc.vector.memset(xr[96:128, 1], 0.0)

                      # q'k' matmuls: 6 fp8 DoubleRow chunks, 1-bank psum each.
                      # Order 0,3,1,4,2,5 so head 0/1 scores (chunks 0+3) can
                      # start after only two copies land.
                      qk_sb = sub_p.tile([128, 6, 512], BF, tag="qksb")
                      for i, cc in enumerate((0, 3, 1, 4, 2, 5)):
                          qp = mm_p.tile([128, 512], F32, tag="mm")
                          nc.tensor.matmul(qp[:],
                                           wqk_s[:, :, 128 * cc:128 * (cc + 1)],
                                           xr[:], start=True, stop=True,
                                           perf_mode=DR)
                          if i % 2 == 0:
                              nc.vector.tensor_scalar_mul(
                                  out=qk_sb[:, cc], in0=qp[:],
                                  scalar1=1.0 / WQK_SC)
                          else:
                              nc.scalar.activation(qk_sb[:, cc], qp[:],
                                                   AF.Copy, scale=1.0 / WQK_SC)

                      # v (token-major, win-major order); 33rd col per head is
                      # the ones column -> softmax denominator rides the av
                      v_sb = sub_p.tile([128, 4, 6, 33], BF, tag="vsb")
                      for vh in range(2):
                          vp = mm_p.tile([128, 512], F32, tag="mm")
                          vpv = vp[:].rearrange("p (a n) -> p a n", a=2)
                          for j in range(2):
                              blk = 2 * vh + j
                              bsl = slice(128 * blk, 128 * (blk + 1))
                              nc.tensor.matmul(vpv[:, j, 0:198],
                                               xr[:, :, bsl], wv_s[:],
                                               start=True, stop=True,
                                               perf_mode=DR)
                          nc.vector.tensor_scalar_mul(
                              out=v_sb[:, 2 * vh:2 * vh + 2],
                              in0=vpv[:, :, 0:198].rearrange(
                                  "p a (h e) -> p a h e", h=6),
                              scalar1=1.0 / WV_SC)
                      nc.vector.memset(v_sb[:, :, :, 32:33], 1.0)

                      # scores per head -> exp (full tile; cross-window
                      # quadrants hold garbage that av never reads)
                      e_t = sub_p.tile([128, 6, 256], BF, tag="et")
                      for h in range(HEADS):
                          ro = 64 * (h % 2)
                          qch, kch = h // 2, 3 + h // 2
                          sc = mm_p.tile([128, 512], F32, tag="mm")
                          for blk in range(NBLK):
                              c0, c1 = 128 * blk, 128 * blk + 64
                              d0 = 64 * blk
                              nc.tensor.matmul(
                                  sc[0:64, d0:d0 + 64],
                                  qk_sb[ro:ro + 64, kch, c0:c0 + 64],
                                  qk_sb[ro:ro + 64, qch, c0:c0 + 64],
                                  start=True, stop=True)
                              nc.tensor.matmul(
                                  sc[64:128, d0:d0 + 64],
                                  qk_sb[ro:ro + 64, kch, c1:c1 + 64],
                                  qk_sb[ro:ro + 64, qch, c1:c1 + 64],
                                  start=True, stop=True)
                          nc.scalar.activation(e_t[:, h], sc[:, 0:256], AF.Exp)

                      # av token-major: out[n, 33] per (head, blk, win) from
                      # the window-diagonal e slices
                      avt = []
                      for i in range(2):
                          av_t = mm_p.tile([128, 512], F32, tag="mm",
                                           name=f"avt{i}")
                          avt.append(av_t[:].rearrange("p (a n) -> p a n", a=2))
                      for h in range(HEADS):
                          for blk in range(NBLK):
                              for wn in range(2):
                                  po = 64 * wn
                                  nc.tensor.matmul(
                                      avt[blk // 2][po:po + 64, blk % 2,
                                                    33 * h:33 * h + 33],
                                      e_t[po:po + 64, h,
                                          64 * blk:64 * blk + 64],
                                      v_sb[po:po + 64, blk, h],
                                      start=True, stop=True)

                      # normalize in token-major (reciprocal over 12 denoms
                      # per partition), then PE-transpose back to feature-major
                      att = sub_p.tile([128, 4, 256], BF, tag="att")
                      for i in range(2):
                          avv = avt[i][:, :, 0:198].rearrange(
                              "p a (h e) -> p a h e", h=6)
                          rec = sub_p.tile([128, 2, 6, 1], F32, tag=f"rec{i}",
                                           name=f"rec{i}")
                          nc.vector.reciprocal(rec[:], avv[:, :, :, 32:33])
                          nc.vector.tensor_tensor(
                              att[:, 2 * i:2 * i + 2, 0:192].rearrange(
                                  "p a (h e) -> p a h e", h=6),
                              avv[:, :, :, 0:32],
                              rec[:].to_broadcast([128, 2, 6, 32]), ALU.mult)

                      at1_ps = av_p.tile([128, 512], BF, tag="atp1")
                      at2_pt = av_p.tile([64, 512], BF, tag="atp2")
                      at2_ps = at2_pt[0:64, :]
                      for blk in range(NBLK):
                          bsl = slice(128 * blk, 128 * (blk + 1))
                          nc.tensor.transpose(at1_ps[:, bsl],
                                              att[:, blk, 0:128], ident[:])
                          nc.tensor.transpose(at2_ps[:, bsl],
                                              att[:, blk, 128:192],
                                              ident[:])
                      at1 = sub_p.tile([128, 512], BF, tag="at1")
                      nc.vector.tensor_copy(out=at1[:], in_=at1_ps[:])
                      at2 = sub_p.tile([64, 512], BF, tag="at2")
                      nc.scalar.activation(at2[:], at2_ps, AF.Copy)

                      # proj
                      pr = mm_p.tile([128, 512], F32, tag="mm")
                      pr2t = mm_p.tile([128, 512], F32, tag="mm")
                      pr2 = pr2t[0:64, :]
                      nc.tensor.matmul(pr[:], wproj1[:, 0:128], at1[:],
                                       start=True, stop=False)
                      nc.tensor.matmul(pr[:], wproj2[:, 0:128], at2[:],
                                       start=False, stop=True)
                      nc.tensor.matmul(pr2, wproj1[:, 128:192], at1[:],
                                       start=True, stop=False)
                      nc.tensor.matmul(pr2, wproj2[:, 128:192], at2[:],
                                       start=False, stop=True)

                      # residual 1 (block-token order -> row-major reorder)
                      ord1 = pr[:].rearrange("p (g v r w) -> p r g v w",
                                             g=4, v=2, r=8)
                      ord2 = pr2.rearrange("p (g v r w) -> p r g v w",
                                           g=4, v=2, r=8)
                      x5 = "p r (g v w) -> p r g v w"
                      nc.vector.tensor_tensor(
                          xh1[sl].rearrange(x5, g=4, v=2),
                          xa[sl].rearrange(x5, g=4, v=2), ord1, ALU.add)
                      nc.vector.tensor_tensor(
                          xh2_[sl].rearrange(x5, g=4, v=2),
                          xb[sl].rearrange(x5, g=4, v=2), ord2, ALU.add)

                  # ---------- LN2 r ----------
                  r2, fa2, la2 = ln_r(xh1, xh2_, "2")
                  add_dep_helper(fa2.ins, la.ins, sync=False,
                                 reason="act-set order")

                  # ---------- MLP per subtile ----------
                  prev_gelu = la2
                  for s in range(NSUB):
                      sl = (slice(None), slice(None), slice(64 * s, 64 * s + 64))
                      rsl = r2[:, 512 * s:512 * (s + 1)].rearrange(
                          "p (r w) -> p r w", r=8)
                      # y in fp8 DoubleRow [128, 2, 512], contraction padded
                      # to 256: slot0 = ch 0:128; slot1 = [ch 128:192, dummy]
                      y = sub_p.tile([128, 2, 512], F8, tag="y")
                      RW = "p (r w) -> p r w"
                      nc.vector.tensor_tensor(
                          y[:, 0].rearrange(RW, r=8), xh1[sl], rsl, ALU.mult)
                      nc.vector.tensor_tensor(
                          y[0:64, 1].rearrange(RW, r=8),
                          xh2_[sl], rsl[0:64], ALU.mult)
                      nc.vector.memset(y[64:128, 1], 0.0)

                      h_sb = sub_p.tile([128, 6, 512], F8, tag="hsb")
                      for cc in range(6):
                          fp = mm_p.tile([128, 512], F32, tag="mm")
                          nc.tensor.matmul(fp[:],
                                           wfc1_s[:, :, 128 * cc:128 * (cc + 1)],
                                           y[:], start=True, stop=True,
                                           perf_mode=DR)
                          g = nc.scalar.activation(h_sb[:, cc], fp[:], AF.Gelu,
                                                   scale=1.0 / WFC1_SC)
                          add_dep_helper(g.ins, prev_gelu.ins, sync=False,
                                         reason="act-set order")
                          prev_gelu = g

                      f2 = mm_p.tile([128, 512], F32, tag="mm")
                      f22t = mm_p.tile([128, 512], F32, tag="mm")
                      f22 = f22t[0:64, :]
                      for i in range(3):
                          nc.tensor.matmul(f2[:], wfc2s[:, 2 * i:2 * i + 2, 0:128],
                                           h_sb[:, 2 * i:2 * i + 2],
                                           start=(i == 0), stop=(i == 2),
                                           perf_mode=DR)
                      for i in range(3):
                          nc.tensor.matmul(f22, wfc2s[:, 2 * i:2 * i + 2, 128:192],
                                           h_sb[:, 2 * i:2 * i + 2],
                                           start=(i == 0), stop=(i == 2),
                                           perf_mode=DR)

                      nc.vector.scalar_tensor_tensor(
                          out1[sl], f2[:].rearrange("p (r w) -> p r w", r=8),
                          1.0 / WFC2_SC, xh1[sl], ALU.mult, ALU.add)
                      nc.vector.scalar_tensor_tensor(
                          out2[sl], f22.rearrange("p (r w) -> p r w", r=8),
                          1.0 / WFC2_SC, xh2_[sl], ALU.mult, ALU.add)
                  prev_act = prev_gelu

                  nc.sync.dma_start(out=out_d[b, 0:128, rows, :], in_=out1[:])
                  nc.sync.dma_start(out=out_d[b, 128:192, rows, :], in_=out2[:])

    fix_sem_range_clear(nc)
    fix_waits(nc)
    bad = audit_waits(nc)
    assert not bad, f"wait audit: {len(bad)} violations: {bad[:3]}"
    return nc


_CACHED = None


def _get_nc():
    global _CACHED
    if _CACHED is None:
        _CACHED = _build_nc()
    return _CACHED


def _make_in_maps(x, bpe_encodings, weights):
    wqk, wv, wproj, wfc1, wfc2 = weights
    wones = np.full((C, 128), 1.0 / C, dtype=_BF16)
    ident = np.eye(128, dtype=_BF16)
    xb = np.asarray(x, _F32).astype(_BF16)
    bb = np.asarray(bpe_encodings, _F32).astype(_BF16)
    in_maps = []
    for s in range(NCORES):
        rows = slice(s * HSH, (s + 1) * HSH)
        in_maps.append({
            "x": np.ascontiguousarray(xb[:, :, rows, :]),
            "bpe": np.ascontiguousarray(bb[:, :, rows, :]),
            "wqk": wqk, "wv": wv, "wproj": wproj,
            "wfc1": wfc1, "wfc2": wfc2, "wones": wones, "ident": ident,
        })
    return in_maps


def kernel(x, bpe_encodings, ln1_w, ln1_b, qkv_w, qkv_b, bpe_w, bpe_b,
           proj_w, proj_b, ln2_w, ln2_b, fc1_w, fc1_b, fc2_w, fc2_b,
           **_kw):
    from concourse.bass_utils import run_bass_kernel_spmd

    weights = _prep_weights(
        np.asarray(ln1_w, _F32), np.asarray(ln1_b, _F32),
        np.asarray(qkv_w, _F32), np.asarray(qkv_b, _F32),
        np.asarray(bpe_w, _F32), np.asarray(bpe_b, _F32),
        np.asarray(proj_w, _F32), np.asarray(proj_b, _F32),
        np.asarray(ln2_w, _F32), np.asarray(ln2_b, _F32),
        np.asarray(fc1_w, _F32), np.asarray(fc1_b, _F32),
        np.asarray(fc2_w, _F32), np.asarray(fc2_b, _F32))
    in_maps = _make_in_maps(x, bpe_encodings, weights)

    nc = _get_nc()
    res = run_bass_kernel_spmd(nc, in_maps, core_ids=list(range(NCORES)))
    out = np.empty((B, C, H, W), dtype=np.float32)
    for s in range(NCORES):
        out[:, :, s * HSH:(s + 1) * HSH, :] = res.results[s]["out"]
    return out
